# revision 10
# baseline (speedup 1.0000x reference)
"""GroupSupConLoss on 8 Trainium2 NeuronCores.

loss = mean over anchors i of (logsumexp_{j!=i}(sim[i,j]) - mean_{j pos}(sim[i,j]))
with sim = E @ E.T / tau.

Device does the O(B^2 D) part: each core owns 1024 rows of the similarity
matrix and computes Z[i] = sum_{j != i} exp(sim[i,j]) via a bf16 GEMM with a
fused exp+row-sum epilogue on the scalar engine (accum_out).

Host does the O(B D) part: positives via the group-sum identity
  sum_pos[i] = (<e_i, G[label_i]> - <e_i, e_i>) / tau,  G[c] = sum of e_j with label c
plus counts, logs, and the final anchor mean (float64).

Sharding trick: each core receives E^T with columns ROTATED so that its own
1024-row block sits at columns 0..1023. The column-sum Z is permutation
invariant, and the diagonal block then sits at a core-independent position,
so one identical SPMD program runs on all 8 cores (diag masked by adding
-1e30*I before the exp). No collectives; host sums the 8 partial outputs.
"""

import numpy as np
import ml_dtypes

import concourse.bass as bass
import concourse.bacc as bacc
import concourse.mybir as mybir
from concourse.bass_utils import run_bass_kernel_spmd
from concourse.tile import TileContext

B = 8192          # batch
D = 1024          # embed dim
NCORES = 8
RPC = B // NCORES  # rows per core = 1024
GCOLS = 2048       # columns per streamed group (4 PSUM banks)
NG = B // GCOLS    # 4 groups
NK = D // 128      # 8 contraction chunks
NRT = RPC // 128   # 8 row tiles per core
NSUB = GCOLS // 512  # 4 matmul column sub-tiles per group
TAU = 0.1
NEG_BIG = -1.0e30

_NC_CACHE = {}


def _build_nc():
    nc = bacc.Bacc(None, target_bir_lowering=False)
    etrot = nc.declare_dram_parameter(
        "etrot", [D, B], mybir.dt.bfloat16, isOutput=False
    )
    ident = nc.declare_dram_parameter(
        "ident", [128, 128], mybir.dt.bfloat16, isOutput=False
    )
    negrhs = nc.declare_dram_parameter(
        "negrhs", [128, GCOLS], mybir.dt.bfloat16, isOutput=False
    )
    zout = nc.declare_dram_parameter(
        "zout", [128, NRT], mybir.dt.float32, isOutput=True
    )

    with TileContext(nc) as tc:
        with (
            tc.tile_pool(name="singles", bufs=1) as singles,
            tc.tile_pool(name="rhsp", bufs=2) as rhsp,
            tc.tile_pool(name="psump", bufs=2, space="PSUM") as psump,
            tc.tile_pool(name="expp", bufs=2) as expp,
        ):
            # Core's own row block (= first RPC rotated columns), kept resident
            # as the stationary matmul operand.
            lhsT = []
            for k in range(NK):
                t = singles.tile(
                    [128, RPC], mybir.dt.bfloat16, name=f"lhsT{k}", tag=f"lhsT{k}"
                )
                nc.sync.dma_start(out=t, in_=etrot[k * 128 : (k + 1) * 128, 0:RPC])
                lhsT.append(t)
            ident_sb = singles.tile([128, 128], mybir.dt.bfloat16, name="ident_sb")
            nc.sync.dma_start(out=ident_sb, in_=ident[:, :])
            negrhs_sb = singles.tile([128, GCOLS], mybir.dt.bfloat16, name="negrhs_sb")
            nc.sync.dma_start(out=negrhs_sb, in_=negrhs[:, :])
            acc = singles.tile([128, NRT, NG], mybir.dt.float32, name="acc")
            zt = singles.tile([128, NRT], mybir.dt.float32, name="zt")

            for g in range(NG):
                rhs = []
                for k in range(NK):
                    t = rhsp.tile(
                        [128, GCOLS],
                        mybir.dt.bfloat16,
                        name=f"rhs_{g}_{k}",
                        tag=f"rhs{k}",
                    )
                    nc.sync.dma_start(
                        out=t,
                        in_=etrot[
                            k * 128 : (k + 1) * 128, g * GCOLS : (g + 1) * GCOLS
                        ],
                    )
                    rhs.append(t)
                for rt in range(NRT):
                    ps = psump.tile(
                        [128, GCOLS], mybir.dt.float32, name=f"ps_{g}_{rt}", tag="ps"
                    )
                    for sub in range(NSUB):
                        # Diagonal block: rotated column rt*128+p equals the
                        # global row index of partition p. Mask it by chaining
                        # one extra accumulation matmul: ident.T @ negrhs adds
                        # -1e30 exactly on the diagonal positions.
                        diag_here = g == 0 and sub == rt // 4
                        for k in range(NK):
                            nc.tensor.matmul(
                                ps[:, sub * 512 : (sub + 1) * 512],
                                lhsT[k][:, rt * 128 : (rt + 1) * 128],
                                rhs[k][:, sub * 512 : (sub + 1) * 512],
                                start=(k == 0),
                                stop=(k == NK - 1) and not diag_here,
                            )
                        if diag_here:
                            o = rt % 4
                            nc.tensor.matmul(
                                ps[:, sub * 512 : (sub + 1) * 512],
                                ident_sb,
                                negrhs_sb[:, o * 512 : (o + 1) * 512],
                                start=False,
                                stop=True,
                            )
                    ex = expp.tile(
                        [128, GCOLS], mybir.dt.bfloat16, name=f"ex_{g}_{rt}", tag="ex"
                    )
                    nc.scalar.activation(
                        out=ex,
                        in_=ps,
                        func=mybir.ActivationFunctionType.Exp,
                        scale=1.0 / TAU,
                        accum_out=acc[:, rt, g : g + 1],
                    )
            for rt in range(NRT):
                nc.vector.reduce_sum(
                    zt[:, rt : rt + 1], acc[:, rt, :], axis=mybir.AxisListType.X
                )
            nc.sync.dma_start(out=zout[:, :], in_=zt)
    nc.finalize()
    return nc


def _get_nc():
    if "nc" not in _NC_CACHE:
        _NC_CACHE["nc"] = _build_nc()
    return _NC_CACHE["nc"]


def _make_runner():
    """Build a cached jitted SPMD executor for the bass program (mirrors
    concourse.bass2jax.run_bass_via_pjrt, but reusable across calls without
    retracing)."""
    if "runner" in _NC_CACHE:
        return _NC_CACHE["runner"]

    import jax
    import concourse.mybir as mybir_
    from concourse import bass2jax
    from concourse.bass2jax import _bass_exec_p, partition_id_tensor
    from jax.sharding import Mesh, PartitionSpec, NamedSharding
    from jax.experimental.shard_map import shard_map

    nc = _get_nc()
    bass2jax.install_neuronx_cc_hook()

    partition_name = nc.partition_id_tensor.name if nc.partition_id_tensor else None
    in_names, out_names, out_avals, zero_outs = [], [], [], []
    for alloc in nc.m.functions[0].allocations:
        if not isinstance(alloc, mybir_.MemoryLocationSet):
            continue
        name = alloc.memorylocations[0].name
        if alloc.kind == "ExternalInput":
            if name != partition_name:
                in_names.append(name)
        elif alloc.kind == "ExternalOutput":
            shape = tuple(alloc.tensor_shape)
            dtype = mybir_.dt.np(alloc.dtype)
            out_names.append(name)
            out_avals.append(jax.core.ShapedArray(shape, dtype))
            zero_outs.append(np.zeros(shape, dtype))
    n_params = len(in_names)
    all_in_names = list(in_names) + list(out_names)
    if partition_name is not None:
        all_in_names.append(partition_name)
    donate = tuple(range(n_params, n_params + len(out_avals)))

    def _body(*args):
        operands = list(args)
        if partition_name is not None:
            operands.append(partition_id_tensor())
        outs = _bass_exec_p.bind(
            *operands,
            out_avals=tuple(out_avals),
            in_names=tuple(all_in_names),
            out_names=tuple(out_names),
            lowering_input_output_aliases=(),
            sim_require_finite=True,
            sim_require_nnan=True,
            nc=nc,
        )
        return tuple(outs)

    devices = jax.devices()[:NCORES]
    mesh = Mesh(np.asarray(devices), ("core",))
    spec = PartitionSpec("core")
    sharded = jax.jit(
        shard_map(
            _body,
            mesh=mesh,
            in_specs=(spec,) * (n_params + len(out_avals)),
            out_specs=(spec,) * len(out_names),
            check_rep=False,
        ),
        donate_argnums=donate,
        keep_unused=True,
    )

    def run(in_maps, staged=None):
        """in_maps: list of per-core dicts. staged: optional pre-staged device
        arrays for the concatenated params (skips H2D)."""
        if staged is None:
            concat_in = [
                np.concatenate([np.asarray(m[name]) for m in in_maps], axis=0)
                for name in in_names
            ]
        else:
            concat_in = staged
        concat_zeros = [
            np.zeros((NCORES * z.shape[0], *z.shape[1:]), z.dtype) for z in zero_outs
        ]
        out_arrs = sharded(*concat_in, *concat_zeros)
        return [
            {
                name: np.asarray(out_arrs[i]).reshape(NCORES, *out_avals[i].shape)[c]
                for i, name in enumerate(out_names)
            }
            for c in range(NCORES)
        ]

    run.in_names = in_names
    run.mesh = mesh
    run.spec = spec
    run.sharded = sharded
    run.zero_outs = zero_outs
    _NC_CACHE["runner"] = run
    return run


def _make_in_maps(embeddings_f32: np.ndarray):
    et = np.ascontiguousarray(embeddings_f32.T).astype(ml_dtypes.bfloat16)  # [D, B]
    ident = np.eye(128, dtype=ml_dtypes.bfloat16)
    negrhs = np.zeros((128, GCOLS), dtype=ml_dtypes.bfloat16)
    for o in range(4):
        for q in range(128):
            negrhs[q, o * 512 + o * 128 + q] = NEG_BIG

    in_maps = []
    for c in range(NCORES):
        etrot = np.roll(et, -c * RPC, axis=1)
        in_maps.append(
            {"etrot": np.ascontiguousarray(etrot), "ident": ident, "negrhs": negrhs}
        )
    return in_maps


def _device_Z(embeddings_f32: np.ndarray):
    """Run the 8-core kernel; returns Z[B] = row sums of exp(sim), diag
    excluded."""
    run = _make_runner()
    results = run(_make_in_maps(embeddings_f32))
    Z = np.concatenate(
        [np.asarray(results[c]["zout"]).T.reshape(-1) for c in range(NCORES)]
    )
    return Z


def kernel(embeddings: np.ndarray, labels: np.ndarray) -> np.ndarray:
    E = np.asarray(embeddings, dtype=np.float32)
    labels = np.asarray(labels)

    Z = _device_Z(E)

    # Host epilogue in float64 (O(B*D) work).
    Ef = E.astype(np.float64)
    lse = np.log(Z.astype(np.float64))

    nclass = int(labels.max()) + 1
    counts = np.bincount(labels, minlength=nclass)
    num_pos = counts[labels] - 1
    G = np.zeros((nclass, D), dtype=np.float64)
    np.add.at(G, labels, Ef)
    sum_pos = (
        np.einsum("ij,ij->i", Ef, G[labels]) - np.einsum("ij,ij->i", Ef, Ef)
    ) / TAU
    mean_pos = sum_pos / np.maximum(num_pos, 1)
    has_pos = num_pos > 0
    loss_i = lse - mean_pos
    loss = np.sum(np.where(has_pos, loss_i, 0.0)) / max(int(has_pos.sum()), 1)
    return np.float32(loss)


# revision 11
# speedup vs baseline: 876.4547x; 876.4547x over previous
"""GroupSupConLoss on 8 Trainium2 NeuronCores.

loss = mean over anchors i of (logsumexp_{j!=i}(sim[i,j]) - mean_{j pos}(sim[i,j]))
with sim = E @ E.T / tau.

Device does the O(B^2 D) part: each core owns 1024 rows of the similarity
matrix and computes Z[i] = sum_{j != i} exp(sim[i,j]) via a bf16 GEMM with a
fused exp+row-sum epilogue on the scalar engine (accum_out).

Host does the O(B D) part: positives via the group-sum identity
  sum_pos[i] = (<e_i, G[label_i]> - <e_i, e_i>) / tau,  G[c] = sum of e_j with label c
plus counts, logs, and the final anchor mean (float64).

Sharding trick: each core receives E^T with columns ROTATED so that its own
1024-row block sits at columns 0..1023. The column-sum Z is permutation
invariant, and the diagonal block then sits at a core-independent position,
so one identical SPMD program runs on all 8 cores. The diagonal is masked on
the tensor engine itself: one extra accumulation matmul per diagonal bank
(identity stationary operand x a -1e30 diagonal-block moving operand), so
exp() flushes those elements to 0. No collectives; host sums the 8 partial
outputs.

Structure per core (modeled 240 us/core vs 218.6 us pure-PE floor at bf16):
  - W = resident [128, 8k, 1024] block (cols 0..1023): stationary matmul
    operand for every tile AND the moving operand for region 0. One 3D DMA.
  - 7 streamed column groups of 1024 (one 2 MB 3D DMA each, double-buffered).
  - Per (region, row-tile): PSUM [128, cols] accumulated over 8 k-chunks per
    512-col bank, then one ScalarE exp (scale=1/tau) with accum_out writing
    the row-sum directly; per-region partial sums reduced at the end.
"""

import numpy as np
import ml_dtypes

import concourse.bacc as bacc
import concourse.mybir as mybir
from concourse.tile import TileContext

B = 8192           # batch
D = 1024           # embed dim
NCORES = 8
RPC = B // NCORES  # rows per core = 1024
NK = D // 128      # 8 contraction chunks
NRT = RPC // 128   # 8 row tiles per core
WCOLS = 1024       # resident region (must equal RPC: holds the diagonal)
GCOLS = 1024       # streamed group width
NGRP = (B - WCOLS) // GCOLS
NREG = 1 + NGRP
TAU = 0.1
NEG_BIG = -1.0e30

_NC_CACHE = {}


def _build_nc(reps: int = 1):
    nc = bacc.Bacc(None, target_bir_lowering=False)
    etrot = nc.declare_dram_parameter(
        "etrot", [D, B], mybir.dt.bfloat16, isOutput=False
    )
    ident = nc.declare_dram_parameter(
        "ident", [128, 128], mybir.dt.bfloat16, isOutput=False
    )
    negrhs = nc.declare_dram_parameter(
        "negrhs", [128, 2048], mybir.dt.bfloat16, isOutput=False
    )
    zout = nc.declare_dram_parameter(
        "zout", [128, NRT], mybir.dt.float32, isOutput=True
    )
    et3 = etrot.rearrange("(nk p) c -> p nk c", p=128)

    with TileContext(nc) as tc:
        with (
            tc.tile_pool(name="singles", bufs=1) as singles,
            tc.tile_pool(name="rhsp", bufs=2) as rhsp,
            tc.tile_pool(name="psump", bufs=2, space="PSUM") as psump,
            tc.tile_pool(name="expp", bufs=2) as expp,
        ):
            W = singles.tile([128, NK, WCOLS], mybir.dt.bfloat16, name="W")
            nc.sync.dma_start(out=W[:, :, :], in_=et3[:, :, 0:WCOLS])
            ident_sb = singles.tile([128, 128], mybir.dt.bfloat16, name="ident_sb")
            nc.sync.dma_start(out=ident_sb, in_=ident[:, :])
            negrhs_sb = singles.tile([128, 2048], mybir.dt.bfloat16, name="negrhs_sb")
            nc.sync.dma_start(out=negrhs_sb, in_=negrhs[:, :])
            acc = singles.tile([128, NRT, NREG], mybir.dt.float32, name="acc")
            zt = singles.tile([128, NRT], mybir.dt.float32, name="zt")

            regions = [("W", 0, WCOLS)] + [
                ("G", WCOLS + i * GCOLS, GCOLS) for i in range(NGRP)
            ]
            for rep in range(reps):
                for ri, (kind, col0, cols) in enumerate(regions):
                    if kind == "W":
                        rhs3 = W
                        rcol0 = 0
                    else:
                        rhs3 = rhsp.tile(
                            [128, NK, cols],
                            mybir.dt.bfloat16,
                            name=f"rhs_{rep}_{ri}",
                            tag="rhs",
                        )
                        nc.sync.dma_start(
                            out=rhs3[:, :, :], in_=et3[:, :, col0 : col0 + cols]
                        )
                        rcol0 = col0
                    nsub = cols // 512
                    for rt in range(NRT):
                        ps = psump.tile(
                            [128, cols],
                            mybir.dt.float32,
                            name=f"ps_{rep}_{ri}_{rt}",
                            tag="ps",
                        )
                        for sub in range(nsub):
                            # Diagonal: rotated column rt*128+p is the global
                            # row of partition p; always inside the W region.
                            diag_here = kind == "W" and sub == rt // 4
                            for k in range(NK):
                                nc.tensor.matmul(
                                    ps[:, sub * 512 : (sub + 1) * 512],
                                    W[:, k, rt * 128 : (rt + 1) * 128],
                                    rhs3[:, k, sub * 512 : (sub + 1) * 512],
                                    start=(k == 0),
                                    stop=(k == NK - 1) and not diag_here,
                                )
                            if diag_here:
                                o = rt % 4
                                nc.tensor.matmul(
                                    ps[:, sub * 512 : (sub + 1) * 512],
                                    ident_sb,
                                    negrhs_sb[:, o * 512 : (o + 1) * 512],
                                    start=False,
                                    stop=True,
                                )
                        ex = expp.tile(
                            [128, cols],
                            mybir.dt.bfloat16,
                            name=f"ex_{rep}_{ri}_{rt}",
                            tag="ex",
                        )
                        nc.scalar.activation(
                            out=ex,
                            in_=ps,
                            func=mybir.ActivationFunctionType.Exp,
                            scale=1.0 / TAU,
                            accum_out=acc[:, rt, ri : ri + 1],
                        )
                        if ri == NREG - 1:
                            nc.vector.reduce_sum(
                                zt[:, rt : rt + 1],
                                acc[:, rt, :],
                                axis=mybir.AxisListType.X,
                            )
            nc.sync.dma_start(out=zout[:, :], in_=zt)
    nc.finalize()
    return nc


def _get_nc():
    if "nc" not in _NC_CACHE:
        _NC_CACHE["nc"] = _build_nc()
    return _NC_CACHE["nc"]


def _make_runner(nc=None, key="runner"):
    """Build a cached jitted SPMD executor for the bass program (mirrors
    concourse.bass2jax.run_bass_via_pjrt, but reusable across calls without
    retracing)."""
    if key in _NC_CACHE:
        return _NC_CACHE[key]

    import jax
    import concourse.mybir as mybir_
    from concourse import bass2jax
    from concourse.bass2jax import _bass_exec_p, partition_id_tensor
    from jax.sharding import Mesh, PartitionSpec
    from jax.experimental.shard_map import shard_map

    if nc is None:
        nc = _get_nc()
    bass2jax.install_neuronx_cc_hook()

    partition_name = nc.partition_id_tensor.name if nc.partition_id_tensor else None
    in_names, out_names, out_avals, zero_outs = [], [], [], []
    for alloc in nc.m.functions[0].allocations:
        if not isinstance(alloc, mybir_.MemoryLocationSet):
            continue
        name = alloc.memorylocations[0].name
        if alloc.kind == "ExternalInput":
            if name != partition_name:
                in_names.append(name)
        elif alloc.kind == "ExternalOutput":
            shape = tuple(alloc.tensor_shape)
            dtype = mybir_.dt.np(alloc.dtype)
            out_names.append(name)
            out_avals.append(jax.core.ShapedArray(shape, dtype))
            zero_outs.append(np.zeros(shape, dtype))
    n_params = len(in_names)
    all_in_names = list(in_names) + list(out_names)
    if partition_name is not None:
        all_in_names.append(partition_name)
    donate = tuple(range(n_params, n_params + len(out_avals)))

    def _body(*args):
        operands = list(args)
        if partition_name is not None:
            operands.append(partition_id_tensor())
        outs = _bass_exec_p.bind(
            *operands,
            out_avals=tuple(out_avals),
            in_names=tuple(all_in_names),
            out_names=tuple(out_names),
            lowering_input_output_aliases=(),
            sim_require_finite=True,
            sim_require_nnan=True,
            nc=nc,
        )
        return tuple(outs)

    devices = jax.devices()[:NCORES]
    mesh = Mesh(np.asarray(devices), ("core",))
    spec = PartitionSpec("core")
    sharded = jax.jit(
        shard_map(
            _body,
            mesh=mesh,
            in_specs=(spec,) * (n_params + len(out_avals)),
            out_specs=(spec,) * len(out_names),
            check_rep=False,
        ),
        donate_argnums=donate,
        keep_unused=True,
    )

    def run(in_maps, staged=None):
        """in_maps: list of per-core dicts. staged: optional pre-staged device
        arrays for the concatenated params (skips H2D)."""
        if staged is None:
            concat_in = [
                np.concatenate([np.asarray(m[name]) for m in in_maps], axis=0)
                for name in in_names
            ]
        else:
            concat_in = staged
        concat_zeros = [
            np.zeros((NCORES * z.shape[0], *z.shape[1:]), z.dtype) for z in zero_outs
        ]
        out_arrs = sharded(*concat_in, *concat_zeros)
        return [
            {
                name: np.asarray(out_arrs[i]).reshape(NCORES, *out_avals[i].shape)[c]
                for i, name in enumerate(out_names)
            }
            for c in range(NCORES)
        ]

    run.in_names = in_names
    run.mesh = mesh
    run.spec = spec
    run.sharded = sharded
    run.zero_outs = zero_outs
    _NC_CACHE[key] = run
    return run


def _make_in_maps(embeddings_f32: np.ndarray):
    et = np.ascontiguousarray(embeddings_f32.T).astype(ml_dtypes.bfloat16)  # [D, B]
    ident = np.eye(128, dtype=ml_dtypes.bfloat16)
    negrhs = np.zeros((128, 2048), dtype=ml_dtypes.bfloat16)
    for o in range(4):
        for q in range(128):
            negrhs[q, o * 512 + o * 128 + q] = NEG_BIG

    in_maps = []
    for c in range(NCORES):
        etrot = np.roll(et, -c * RPC, axis=1)
        in_maps.append(
            {"etrot": np.ascontiguousarray(etrot), "ident": ident, "negrhs": negrhs}
        )
    return in_maps


def _device_Z(embeddings_f32: np.ndarray):
    """Run the 8-core kernel; returns Z[B] = row sums of exp(sim), diag
    excluded."""
    run = _make_runner()
    results = run(_make_in_maps(embeddings_f32))
    Z = np.concatenate(
        [np.asarray(results[c]["zout"]).T.reshape(-1) for c in range(NCORES)]
    )
    return Z


def kernel(embeddings: np.ndarray, labels: np.ndarray) -> np.ndarray:
    E = np.asarray(embeddings, dtype=np.float32)
    labels = np.asarray(labels)

    Z = _device_Z(E)

    # Host epilogue in float64 (O(B*D) work).
    Ef = E.astype(np.float64)
    lse = np.log(Z.astype(np.float64))

    nclass = int(labels.max()) + 1
    counts = np.bincount(labels, minlength=nclass)
    num_pos = counts[labels] - 1
    G = np.zeros((nclass, D), dtype=np.float64)
    np.add.at(G, labels, Ef)
    sum_pos = (
        np.einsum("ij,ij->i", Ef, G[labels]) - np.einsum("ij,ij->i", Ef, Ef)
    ) / TAU
    mean_pos = sum_pos / np.maximum(num_pos, 1)
    has_pos = num_pos > 0
    loss_i = lse - mean_pos
    loss = np.sum(np.where(has_pos, loss_i, 0.0)) / max(int(has_pos.sum()), 1)
    return np.float32(loss)
